# revision 25
# baseline (speedup 1.0000x reference)
# Trainium2 Bass kernel for LocLoss: per-sample argmax over a 192x192 cls map,
# gather of loc values at the argmax position, smooth-L1 loss vs a
# center_rate-derived bias, mean-reduced.
#
# Sharding: pure data parallel, batch 256 -> 8 cores x 32 samples.
# Per-core layout: partition p = 4*s + ch holds chunk ch (48 rows) of sample
# s's 192x192 map -- a pure host reshape, no shuffle. The bulk load streams
# over the sync HWDGE queue in 8 in-order slices with the per-row max reduced
# on Vector as each slice lands. The tail then runs at chunk-candidate
# granularity (one candidate per partition, all offsets affine in p):
#   max/max_index over the 48 row-maxes -> winning row rr per chunk
#   two concurrent indirect gathers, both keyed on rr only:
#     cls winning row (128,192) and loc winning row pair (128,384)
#   max_index on the gathered cls row -> column cc
#   one-hot dot extracts the two loc values; smooth-L1 on-device.
# Output per core: (128, 2) = [chunk max value, lossY+lossX]; the host picks
# the best of each sample's 4 chunk candidates (the global argmax) and means.
import numpy as np
from contextlib import ExitStack

import concourse.bass as bass
import concourse.bacc as bacc
import concourse.mybir as mybir
import concourse.tile as tile

B = 256
NCORES = 8
BP = B // NCORES          # 32 samples per core
H = W = 192
MAP = H * W               # 36864
NCHUNK = 4                # chunks per sample -> 128 partitions
ROWS_PER_PART = H // NCHUNK   # 48
CHUNK = ROWS_PER_PART * W     # 9216

# bulk slices (rows per partition); must sum to ROWS_PER_PART. Sized so the
# vector reduce chain trails the last DMA byte by ~1us.
SLICE_ROWS = [10, 9, 8, 7, 6, 5, 2, 1]
assert sum(SLICE_ROWS) == ROWS_PER_PART

F32 = mybir.dt.float32
I32 = mybir.dt.int32
U32 = mybir.dt.uint32
ALU = mybir.AluOpType
AX = mybir.AxisListType


def build_program(with_dbg=False, stage=6):
    nc = bacc.Bacc("TRN2", target_bir_lowering=False, debug=False, num_devices=NCORES)

    # cls as rows of 192: row index for the gather is 48*p + rr
    cls_d = nc.dram_tensor("cls", [128 * ROWS_PER_PART, W], F32,
                           kind="ExternalInput")
    # combined per-row gather payload [locY row | locX row | cls row]:
    # one indirect gather at row 48*p + rr fetches everything the tail needs.
    comb_d = nc.dram_tensor("comb", [128 * ROWS_PER_PART, 3 * W], F32,
                           kind="ExternalInput")
    # aux[p] = [48*p, 191*crY[s] - 48*(p%4), 191*crX[s], 0] (host-folded)
    aux_d = nc.dram_tensor("aux", [128, 4], F32, kind="ExternalInput")
    auxu_d = nc.dram_tensor("auxu", [128, 1], U32, kind="ExternalInput")
    loss_d = nc.dram_tensor("loss", [128, 2], F32, kind="ExternalOutput")
    dbg_d = (nc.dram_tensor("dbg", [128, 8], F32, kind="ExternalOutput")
             if with_dbg else None)

    cls_rows = cls_d[:].rearrange("(p r) c -> p (r c)", p=128)  # (128, 9216)

    with tile.TileContext(nc) as tc:
        with ExitStack() as ctx:
            pool = ctx.enter_context(tc.tile_pool(name="p", bufs=1))

            aux = pool.tile([128, 4], F32, tag="aux")
            nc.scalar.dma_start(aux[:], aux_d[:])
            auxu = pool.tile([128, 1], U32, tag="auxu")
            nc.scalar.dma_start(auxu[:], auxu_d[:])

            iota_i = pool.tile([128, W], I32, tag="iotai")
            iota_f = pool.tile([128, W], F32, tag="iotaf")
            row_max = pool.tile([128, ROWS_PER_PART], F32, tag="rowmax")

            # --- bulk: stream cls in slices over the sync HWDGE queue,
            # reducing each slice's rows on Vector as it lands.
            r0 = 0
            for i, nrows in enumerate(SLICE_ROWS):
                t = pool.tile([128, nrows * W], F32, tag=f"s{i}")
                eng = nc.sync if i == 0 else nc.gpsimd
                eng.dma_start(t[:], cls_rows[:, r0 * W:(r0 + nrows) * W])
                nc.vector.reduce_max(
                    row_max[:, r0:r0 + nrows],
                    t[:].rearrange("p (a c) -> p a c", c=W),
                    axis=AX.X,
                )
                r0 += nrows

            # col iota 0..191 as f32 for the one-hot select -- emitted after
            # the bulk DMAs so it doesn't delay the first slice's descriptors
            nc.gpsimd.iota(iota_i[:], pattern=[[1, W]], base=0,
                           channel_multiplier=0)
            nc.vector.tensor_copy(iota_f[:], iota_i[:])

            if stage <= 1:
                nc.sync.dma_start(loss_d[:], row_max[:, 0:2])

            if stage >= 2:
                # --- per-chunk argmax row
                m8 = pool.tile([128, 8], F32, tag="m8")
                ri8 = pool.tile([128, 8], U32, tag="ri8")
                nc.vector.max(out=m8[:], in_=row_max[:])
                nc.vector.max_index(out=ri8[:], in_max=m8[:], in_values=row_max[:])

                rcf = pool.tile([128, 2], F32, tag="rcf")   # [rr, cc] as f32

                # shared gather row index: 48*p + rr (uint32 add)
                rowu = pool.tile([128, 1], U32, tag="rowu")
                nc.vector.tensor_tensor(rowu[:], ri8[:, 0:1], auxu[:],
                                        op=ALU.add)

                if stage <= 2:
                    nc.sync.dma_start(loss_d[:], rcf[:])

            if stage >= 3:
                # --- two concurrent gathers, both keyed on the winning row
                # one combined gather keyed on the winning row:
                # [locY row | locX row | cls row]
                comb = pool.tile([128, 3 * W], F32, tag="comb")
                nc.gpsimd.indirect_dma_start(
                    out=comb[:],
                    out_offset=None,
                    in_=comb_d[:],
                    in_offset=bass.IndirectOffsetOnAxis(ap=rowu[:, 0:1], axis=0),
                )
                rows_t = comb[:, 2 * W:3 * W]
                nc.vector.tensor_copy(rcf[:, 0:1], ri8[:, 0:1])  # in gather
                if stage <= 3:
                    nc.sync.dma_start(loss_d[:], comb[:, 0:2])

            if stage >= 5:
                cand = pool.tile([128, 2], F32, tag="cand")  # [val, loss sum]
                nc.vector.tensor_copy(cand[:, 0:1], m8[:, 0:1])  # in gathers

                # --- column of the chunk max within the gathered row
                # (max_index faults if a searched value is absent, so re-max
                # over the gathered row rather than reusing m8[1:].)
                rm8 = pool.tile([128, 8], F32, tag="rm8")
                ci8 = pool.tile([128, 8], U32, tag="ci8")
                nc.vector.max(out=rm8[:], in_=rows_t)
                nc.vector.max_index(out=ci8[:], in_max=rm8[:], in_values=rows_t)
                nc.vector.tensor_copy(rcf[:, 1:2], ci8[:, 0:1])

                # --- one-hot dot: loc values at column cc
                # (TensorScalarPtr only allows arithmetic combos like
                # (mult-imm, add-AP); comparisons must be immediate-only.)
                onehot = pool.tile([128, W], F32, tag="onehot")
                nc.vector.tensor_scalar(onehot[:], iota_f[:], -1.0, rcf[:, 1:2],
                                        op0=ALU.mult, op1=ALU.add)  # cc - iota
                nc.vector.tensor_tensor(onehot[:], onehot[:], onehot[:],
                                        op=ALU.mult)                # squared
                nc.vector.tensor_scalar(onehot[:], onehot[:], 0.5, None,
                                        op0=ALU.is_lt)              # one-hot
                scr = pool.tile([128, W], F32, tag="scr")
                scrx = pool.tile([128, W], F32, tag="scrx")
                loc_pos = pool.tile([128, 2], F32, tag="locp")
                nc.vector.tensor_tensor(scr[:], onehot[:], comb[:, 0:W],
                                        op=ALU.mult)
                nc.vector.tensor_tensor(scrx[:], onehot[:], comb[:, W:2 * W],
                                        op=ALU.mult)
                nc.vector.tensor_reduce(loc_pos[:, 0:1], scr[:], axis=AX.X,
                                        op=ALU.add)
                nc.vector.tensor_reduce(loc_pos[:, 1:2], scrx[:], axis=AX.X,
                                        op=ALU.add)
                if stage <= 5:
                    nc.sync.dma_start(loss_d[:], loc_pos[:])

            if stage >= 6:
                # d = loc - (191*cr - [r_map, c]) = (loc - aux[:,2:4]) + [rr,cc]
                d2 = pool.tile([128, 2], F32, tag="d2")
                nc.vector.tensor_tensor(d2[:], loc_pos[:], aux[:, 1:3],
                                        op=ALU.subtract)
                nc.vector.tensor_tensor(d2[:], d2[:], rcf[:], op=ALU.add)
                # smooth L1 (beta=1): a=|d|; h=min(a,1); loss = h*(a - 0.5h)
                u2 = pool.tile([128, 2], F32, tag="u2")
                nc.vector.tensor_scalar_mul(u2[:], d2[:], -1.0)
                a2 = pool.tile([128, 2], F32, tag="a2")
                nc.vector.tensor_tensor(a2[:], d2[:], u2[:], op=ALU.max)
                h2 = pool.tile([128, 2], F32, tag="h2")
                nc.vector.tensor_scalar_min(h2[:], a2[:], 1.0)
                t2 = pool.tile([128, 2], F32, tag="t2")
                nc.vector.scalar_tensor_tensor(t2[:], h2[:], -0.5, a2[:],
                                               op0=ALU.mult, op1=ALU.add)
                l2 = pool.tile([128, 2], F32, tag="l2")
                nc.vector.tensor_tensor(l2[:], h2[:], t2[:], op=ALU.mult)
                nc.vector.tensor_tensor(cand[:, 1:2], l2[:, 0:1], l2[:, 1:2],
                                        op=ALU.add)

                nc.sync.dma_start(loss_d[:], cand[:])

            if with_dbg:
                dbg = pool.tile([128, 8], F32, tag="dbg")
                nc.vector.tensor_copy(dbg[:, 0:1], m8[:, 0:1])
                nc.vector.tensor_copy(dbg[:, 1:3], rcf[:])
                nc.vector.tensor_copy(dbg[:, 3:4], off_f[:])
                nc.vector.tensor_copy(dbg[:, 4:6], loc_pos[:])
                nc.vector.tensor_copy(dbg[:, 6:8], l2[:])
                nc.sync.dma_start(dbg_d[:], dbg[:])

    nc.compile()
    return nc


_NC_CACHE = {}


def _get_program(with_dbg=False):
    if with_dbg not in _NC_CACHE:
        _NC_CACHE[with_dbg] = build_program(with_dbg)
    return _NC_CACHE[with_dbg]


_P = np.arange(128)
_AUX_C0 = (18432.0 * _P).astype(np.float32)        # exact in f32 (< 2^24)
_AUX_C1 = (48.0 * _P).astype(np.float32)
_AUX_ROW = (48.0 * (_P % 4)).astype(np.float32)


def make_in_maps(cls_input, loc_input, center_rate):
    # p = 4*s + ch: pure reshape, rows 48*ch..48*ch+47 of sample s -> part p
    cls = np.ascontiguousarray(np.asarray(cls_input, dtype=np.float32)).reshape(
        NCORES, 128 * ROWS_PER_PART, W)
    loc = np.asarray(loc_input, dtype=np.float32).reshape(B, 2, H, W)
    comb = np.empty((B, H, 3 * W), dtype=np.float32)
    comb[:, :, 0:W] = loc[:, 0]
    comb[:, :, W:2 * W] = loc[:, 1]
    comb[:, :, 2 * W:3 * W] = np.asarray(
        cls_input, dtype=np.float32).reshape(B, H, W)
    comb = comb.reshape(NCORES, 128 * ROWS_PER_PART, 3 * W)
    cr = np.asarray(center_rate, dtype=np.float32).reshape(NCORES, BP, 2)
    crr = np.repeat(cr, NCHUNK, axis=1)            # (NCORES, 128, 2)
    aux = np.zeros((NCORES, 128, 4), dtype=np.float32)
    aux[:, :, 0] = _AUX_C1
    aux[:, :, 1] = np.float32(191.0) * crr[:, :, 0] - _AUX_ROW
    aux[:, :, 2] = np.float32(191.0) * crr[:, :, 1]
    auxu = (48 * _P).astype(np.uint32).reshape(128, 1)
    return [
        {"cls": cls[c], "comb": comb[c], "aux": aux[c], "auxu": auxu}
        for c in range(NCORES)
    ]


def kernel(cls_input, loc_input, center_rate, _trace=False, _results_out=None,
           _dbg=False):
    from concourse.bass_utils import run_bass_kernel_spmd

    nc = _get_program(_dbg)
    in_maps = make_in_maps(cls_input, loc_input, center_rate)
    res = run_bass_kernel_spmd(nc, in_maps, list(range(NCORES)), trace=_trace)
    if _results_out is not None:
        _results_out.append(res)
    out = np.stack([r["loss"] for r in res.results], axis=0)  # (8, 128, 2)
    vals = out[:, :, 0].reshape(B, NCHUNK)
    ls = out[:, :, 1].reshape(B, NCHUNK)
    sel = np.argmax(vals, axis=1)
    loss_sum = ls[np.arange(B), sel]
    return np.float32(np.sum(loss_sum, dtype=np.float64) / (2 * B))


# revision 26
# speedup vs baseline: 1.0776x; 1.0776x over previous
# Trainium2 Bass kernel for LocLoss: per-sample argmax over a 192x192 cls map,
# gather of loc values at the argmax position, smooth-L1 loss vs a
# center_rate-derived bias, mean-reduced.
#
# Sharding: pure data parallel, batch 256 -> 8 cores x 32 samples.
# Per-core layout: partition p = 4*s + ch holds chunk ch (48 rows) of sample
# s's 192x192 map -- a pure host reshape, no shuffle. The bulk load streams
# over the sync HWDGE queue in 8 in-order slices with the per-row max reduced
# on Vector as each slice lands. The tail then runs at chunk-candidate
# granularity (one candidate per partition, all offsets affine in p):
#   max/max_index over the 48 row-maxes -> winning row rr per chunk
#   two concurrent indirect gathers, both keyed on rr only:
#     cls winning row (128,192) and loc winning row pair (128,384)
#   max_index on the gathered cls row -> column cc
#   one-hot dot extracts the two loc values; smooth-L1 on-device.
# Output per core: (128, 2) = [chunk max value, lossY+lossX]; the host picks
# the best of each sample's 4 chunk candidates (the global argmax) and means.
import numpy as np
from contextlib import ExitStack

import concourse.bass as bass
import concourse.bacc as bacc
import concourse.mybir as mybir
import concourse.tile as tile

B = 256
NCORES = 8
BP = B // NCORES          # 32 samples per core
H = W = 192
MAP = H * W               # 36864
NCHUNK = 4                # chunks per sample -> 128 partitions
ROWS_PER_PART = H // NCHUNK   # 48
CHUNK = ROWS_PER_PART * W     # 9216

# bulk slices (rows per partition); must sum to ROWS_PER_PART. Sized so the
# vector reduce chain trails the last DMA byte by ~1us.
SLICE_ROWS = [10, 9, 8, 7, 6, 5, 2, 1]
assert sum(SLICE_ROWS) == ROWS_PER_PART

F32 = mybir.dt.float32
I32 = mybir.dt.int32
U32 = mybir.dt.uint32
ALU = mybir.AluOpType
AX = mybir.AxisListType


def build_program(with_dbg=False, stage=6):
    nc = bacc.Bacc("TRN2", target_bir_lowering=False, debug=False, num_devices=NCORES)

    # cls as rows of 192: row index for the gather is 48*p + rr
    cls_d = nc.dram_tensor("cls", [128 * ROWS_PER_PART, W], F32,
                           kind="ExternalInput")
    # combined per-row gather payload [locY row | locX row | cls row]:
    # one indirect gather at row 48*p + rr fetches everything the tail needs.
    comb_d = nc.dram_tensor("comb", [128 * ROWS_PER_PART, 3 * W], F32,
                           kind="ExternalInput")
    # aux[p] = [48*p, 191*crY[s] - 48*(p%4), 191*crX[s], 0] (host-folded)
    aux_d = nc.dram_tensor("aux", [128, 4], F32, kind="ExternalInput")
    auxu_d = nc.dram_tensor("auxu", [128, 1], U32, kind="ExternalInput")
    loss_d = nc.dram_tensor("loss", [128, 2], F32, kind="ExternalOutput")
    dbg_d = (nc.dram_tensor("dbg", [128, 8], F32, kind="ExternalOutput")
             if with_dbg else None)

    cls_rows = cls_d[:].rearrange("(p r) c -> p (r c)", p=128)  # (128, 9216)

    with tile.TileContext(nc) as tc:
        with ExitStack() as ctx:
            pool = ctx.enter_context(tc.tile_pool(name="p", bufs=1))

            aux = pool.tile([128, 4], F32, tag="aux")
            nc.scalar.dma_start(aux[:], aux_d[:])
            auxu = pool.tile([128, 1], U32, tag="auxu")
            nc.scalar.dma_start(auxu[:], auxu_d[:])

            iota_i = pool.tile([128, W], I32, tag="iotai")
            iota_f = pool.tile([128, W], F32, tag="iotaf")
            row_max = pool.tile([128, ROWS_PER_PART], F32, tag="rowmax")

            # --- bulk: stream cls in slices over the sync HWDGE queue,
            # reducing each slice's rows on Vector as it lands.
            r0 = 0
            for i, nrows in enumerate(SLICE_ROWS):
                t = pool.tile([128, nrows * W], F32, tag=f"s{i}")
                nc.gpsimd.dma_start(t[:], cls_rows[:, r0 * W:(r0 + nrows) * W])
                nc.vector.reduce_max(
                    row_max[:, r0:r0 + nrows],
                    t[:].rearrange("p (a c) -> p a c", c=W),
                    axis=AX.X,
                )
                r0 += nrows

            # col iota 0..191 as f32 for the one-hot select -- emitted after
            # the bulk DMAs so it doesn't delay the first slice's descriptors
            nc.gpsimd.iota(iota_i[:], pattern=[[1, W]], base=0,
                           channel_multiplier=0)
            nc.vector.tensor_copy(iota_f[:], iota_i[:])

            if stage <= 1:
                nc.sync.dma_start(loss_d[:], row_max[:, 0:2])

            if stage >= 2:
                # --- per-chunk argmax row
                m8 = pool.tile([128, 8], F32, tag="m8")
                ri8 = pool.tile([128, 8], U32, tag="ri8")
                nc.vector.max(out=m8[:], in_=row_max[:])
                nc.vector.max_index(out=ri8[:], in_max=m8[:], in_values=row_max[:])

                rcf = pool.tile([128, 2], F32, tag="rcf")   # [rr, cc] as f32

                # shared gather row index: 48*p + rr (uint32 add)
                rowu = pool.tile([128, 1], U32, tag="rowu")
                nc.vector.tensor_tensor(rowu[:], ri8[:, 0:1], auxu[:],
                                        op=ALU.add)

                if stage <= 2:
                    nc.sync.dma_start(loss_d[:], rcf[:])

            if stage >= 3:
                # --- two concurrent gathers, both keyed on the winning row
                # one combined gather keyed on the winning row:
                # [locY row | locX row | cls row]
                comb = pool.tile([128, 3 * W], F32, tag="comb")
                nc.gpsimd.indirect_dma_start(
                    out=comb[:],
                    out_offset=None,
                    in_=comb_d[:],
                    in_offset=bass.IndirectOffsetOnAxis(ap=rowu[:, 0:1], axis=0),
                )
                rows_t = comb[:, 2 * W:3 * W]
                nc.vector.tensor_copy(rcf[:, 0:1], ri8[:, 0:1])  # in gather
                if stage <= 3:
                    nc.sync.dma_start(loss_d[:], comb[:, 0:2])

            if stage >= 5:
                cand = pool.tile([128, 2], F32, tag="cand")  # [val, loss sum]
                nc.vector.tensor_copy(cand[:, 0:1], m8[:, 0:1])  # in gathers

                # --- column of the chunk max within the gathered row
                # (max_index faults if a searched value is absent, so re-max
                # over the gathered row rather than reusing m8[1:].)
                rm8 = pool.tile([128, 8], F32, tag="rm8")
                ci8 = pool.tile([128, 8], U32, tag="ci8")
                nc.vector.max(out=rm8[:], in_=rows_t)
                nc.vector.max_index(out=ci8[:], in_max=rm8[:], in_values=rows_t)
                nc.vector.tensor_copy(rcf[:, 1:2], ci8[:, 0:1])

                # --- one-hot dot: loc values at column cc
                # (TensorScalarPtr only allows arithmetic combos like
                # (mult-imm, add-AP); comparisons must be immediate-only.)
                onehot = pool.tile([128, W], F32, tag="onehot")
                nc.vector.tensor_scalar(onehot[:], iota_f[:], -1.0, rcf[:, 1:2],
                                        op0=ALU.mult, op1=ALU.add)  # cc - iota
                nc.vector.tensor_tensor(onehot[:], onehot[:], onehot[:],
                                        op=ALU.mult)                # squared
                nc.vector.tensor_scalar(onehot[:], onehot[:], 0.5, None,
                                        op0=ALU.is_lt)              # one-hot
                scr = pool.tile([128, W], F32, tag="scr")
                scrx = pool.tile([128, W], F32, tag="scrx")
                loc_pos = pool.tile([128, 2], F32, tag="locp")
                nc.vector.tensor_tensor(scr[:], onehot[:], comb[:, 0:W],
                                        op=ALU.mult)
                nc.vector.tensor_tensor(scrx[:], onehot[:], comb[:, W:2 * W],
                                        op=ALU.mult)
                nc.vector.tensor_reduce(loc_pos[:, 0:1], scr[:], axis=AX.X,
                                        op=ALU.add)
                nc.vector.tensor_reduce(loc_pos[:, 1:2], scrx[:], axis=AX.X,
                                        op=ALU.add)
                if stage <= 5:
                    nc.sync.dma_start(loss_d[:], loc_pos[:])

            if stage >= 6:
                # d = loc - (191*cr - [r_map, c]) = (loc - aux[:,2:4]) + [rr,cc]
                d2 = pool.tile([128, 2], F32, tag="d2")
                nc.vector.tensor_tensor(d2[:], loc_pos[:], aux[:, 1:3],
                                        op=ALU.subtract)
                nc.vector.tensor_tensor(d2[:], d2[:], rcf[:], op=ALU.add)
                # smooth L1 (beta=1): a=|d|; h=min(a,1); loss = h*(a - 0.5h)
                u2 = pool.tile([128, 2], F32, tag="u2")
                nc.vector.tensor_scalar_mul(u2[:], d2[:], -1.0)
                a2 = pool.tile([128, 2], F32, tag="a2")
                nc.vector.tensor_tensor(a2[:], d2[:], u2[:], op=ALU.max)
                h2 = pool.tile([128, 2], F32, tag="h2")
                nc.vector.tensor_scalar_min(h2[:], a2[:], 1.0)
                t2 = pool.tile([128, 2], F32, tag="t2")
                nc.vector.scalar_tensor_tensor(t2[:], h2[:], -0.5, a2[:],
                                               op0=ALU.mult, op1=ALU.add)
                l2 = pool.tile([128, 2], F32, tag="l2")
                nc.vector.tensor_tensor(l2[:], h2[:], t2[:], op=ALU.mult)
                nc.vector.tensor_tensor(cand[:, 1:2], l2[:, 0:1], l2[:, 1:2],
                                        op=ALU.add)

                nc.sync.dma_start(loss_d[:], cand[:])

            if with_dbg:
                dbg = pool.tile([128, 8], F32, tag="dbg")
                nc.vector.tensor_copy(dbg[:, 0:1], m8[:, 0:1])
                nc.vector.tensor_copy(dbg[:, 1:3], rcf[:])
                nc.vector.tensor_copy(dbg[:, 3:4], off_f[:])
                nc.vector.tensor_copy(dbg[:, 4:6], loc_pos[:])
                nc.vector.tensor_copy(dbg[:, 6:8], l2[:])
                nc.sync.dma_start(dbg_d[:], dbg[:])

    nc.compile()
    return nc


_NC_CACHE = {}


def _get_program(with_dbg=False):
    if with_dbg not in _NC_CACHE:
        _NC_CACHE[with_dbg] = build_program(with_dbg)
    return _NC_CACHE[with_dbg]


_P = np.arange(128)
_AUX_C0 = (18432.0 * _P).astype(np.float32)        # exact in f32 (< 2^24)
_AUX_C1 = (48.0 * _P).astype(np.float32)
_AUX_ROW = (48.0 * (_P % 4)).astype(np.float32)


def make_in_maps(cls_input, loc_input, center_rate):
    # p = 4*s + ch: pure reshape, rows 48*ch..48*ch+47 of sample s -> part p
    cls = np.ascontiguousarray(np.asarray(cls_input, dtype=np.float32)).reshape(
        NCORES, 128 * ROWS_PER_PART, W)
    loc = np.asarray(loc_input, dtype=np.float32).reshape(B, 2, H, W)
    comb = np.empty((B, H, 3 * W), dtype=np.float32)
    comb[:, :, 0:W] = loc[:, 0]
    comb[:, :, W:2 * W] = loc[:, 1]
    comb[:, :, 2 * W:3 * W] = np.asarray(
        cls_input, dtype=np.float32).reshape(B, H, W)
    comb = comb.reshape(NCORES, 128 * ROWS_PER_PART, 3 * W)
    cr = np.asarray(center_rate, dtype=np.float32).reshape(NCORES, BP, 2)
    crr = np.repeat(cr, NCHUNK, axis=1)            # (NCORES, 128, 2)
    aux = np.zeros((NCORES, 128, 4), dtype=np.float32)
    aux[:, :, 0] = _AUX_C1
    aux[:, :, 1] = np.float32(191.0) * crr[:, :, 0] - _AUX_ROW
    aux[:, :, 2] = np.float32(191.0) * crr[:, :, 1]
    auxu = (48 * _P).astype(np.uint32).reshape(128, 1)
    return [
        {"cls": cls[c], "comb": comb[c], "aux": aux[c], "auxu": auxu}
        for c in range(NCORES)
    ]


def kernel(cls_input, loc_input, center_rate, _trace=False, _results_out=None,
           _dbg=False):
    from concourse.bass_utils import run_bass_kernel_spmd

    nc = _get_program(_dbg)
    in_maps = make_in_maps(cls_input, loc_input, center_rate)
    res = run_bass_kernel_spmd(nc, in_maps, list(range(NCORES)), trace=_trace)
    if _results_out is not None:
        _results_out.append(res)
    out = np.stack([r["loss"] for r in res.results], axis=0)  # (8, 128, 2)
    vals = out[:, :, 0].reshape(B, NCHUNK)
    ls = out[:, :, 1].reshape(B, NCHUNK)
    sel = np.argmax(vals, axis=1)
    loss_sum = ls[np.arange(B), sel]
    return np.float32(np.sum(loss_sum, dtype=np.float64) / (2 * B))
